# revision 2
# baseline (speedup 1.0000x reference)
"""Trainium2 8-core tensor-parallel transformer block (RMSNorm + RoPE causal
attention + SwiGLU FFN).

Sharding (SPMD, identical program on 8 cores, per-core data via in_maps):
  - attention: heads sharded (2 heads/core); q/k/v projections row-sharded
  - o-projection and FFN down-projection: token-sharded after an AllToAll
  - FFN up (w1/w3): dff-sharded
  - norms: token-sharded, AllGather of normalized activations (bf16)

Collectives: AG(h1) -> attn -> A2A(head outputs) -> o-proj(own tokens)
             -> AG(h2) -> ffn up -> A2A(activations) -> down-proj(own tokens)

All matmuls bf16 x bf16 -> fp32 PSUM. Residual spine fp32.
RoPE is applied as q_rot = X*C + X_swap*S where X_swap comes from a second
matmul with row-swapped weights (avoids partition-crossing vector ops);
q/k rows are host-permuted to [h0-even, h0-odd, h1-even, h1-odd], which
leaves q.k dot products unchanged.
Softmax skips the running max (|scores| < 4 for this problem's scale) and
gets its denominator for free from a ones-column appended to V.
"""

from contextlib import ExitStack

import numpy as np
import ml_dtypes

import concourse.mybir as mybir
import concourse.tile as tile
from concourse import bacc
from concourse.bass import ds, ts
from concourse.bass_utils import run_bass_kernel_spmd

B, S, D, H, DK, DFF = 2, 2048, 1024, 16, 64, 4096
THETA, EPS = 10000.0, 1e-5
N = 8            # cores
T = B * S // N   # tokens per core (512)
HPC = H // N     # heads per core (2)
DFFC = DFF // N  # dff per core (512)

F32 = mybir.dt.float32
BF16 = mybir.dt.bfloat16
BF16NP = ml_dtypes.bfloat16
AF = mybir.ActivationFunctionType

_NC = None


def _build():
    nc = bacc.Bacc("TRN2", target_bir_lowering=False)

    # ---- I/O ----
    x_in = nc.dram_tensor("x", [T, D], F32, kind="ExternalInput")
    wqT = nc.dram_tensor("wqT", [D, 128], BF16, kind="ExternalInput")
    wkT = nc.dram_tensor("wkT", [D, 128], BF16, kind="ExternalInput")
    perm_in = nc.dram_tensor("rope_perm", [128, 128], BF16, kind="ExternalInput")
    ident_in = nc.dram_tensor("ident", [128, 128], BF16, kind="ExternalInput")
    wvT = nc.dram_tensor("wvT", [D, 128], BF16, kind="ExternalInput")
    woT = nc.dram_tensor("woT", [D, D], BF16, kind="ExternalInput")
    w1T = nc.dram_tensor("w1T", [D, DFFC], BF16, kind="ExternalInput")
    w3T = nc.dram_tensor("w3T", [D, DFFC], BF16, kind="ExternalInput")
    w2T = nc.dram_tensor("w2T", [DFF, D], BF16, kind="ExternalInput")
    trigC = nc.dram_tensor("trigC", [128, S], F32, kind="ExternalInput")
    trigS = nc.dram_tensor("trigS", [128, S], F32, kind="ExternalInput")
    mask_in = nc.dram_tensor("mask", [128, 128], BF16, kind="ExternalInput")
    out_ext = nc.dram_tensor("out", [T, D], F32, kind="ExternalOutput")

    # ---- internal DRAM ----
    h1_tm = nc.dram_tensor("h1_tm", [T, D], BF16)
    h1t_in = nc.dram_tensor("h1t_in", [D, T], BF16)
    h1t_ag = nc.dram_tensor("h1t_ag", [N * D, T], BF16, addr_space="Shared")
    h2_tm = nc.dram_tensor("h2_tm", [T, D], BF16)
    h2t_in = nc.dram_tensor("h2t_in", [D, T], BF16)
    h2t_ag = nc.dram_tensor("h2t_ag", [N * D, T], BF16, addr_space="Shared")
    o_a2a_in = nc.dram_tensor("o_a2a_in", [N * 128, T], BF16)
    o_a2a_out = nc.dram_tensor("o_a2a_out", [N * 128, T], BF16)
    s_a2a_in_a = nc.dram_tensor("s_a2a_in_a", [N * DFFC // 2, T], BF16)
    s_a2a_in_b = nc.dram_tensor("s_a2a_in_b", [N * DFFC // 2, T], BF16)
    s_a2a_out_a = nc.dram_tensor("s_a2a_out_a", [N * DFFC // 2, T], BF16)
    s_a2a_out_b = nc.dram_tensor("s_a2a_out_b", [N * DFFC // 2, T], BF16)

    rg = [list(range(N))]

    with tile.TileContext(nc) as tc, ExitStack() as stack:
        consts = stack.enter_context(tc.tile_pool(name="consts", bufs=1))
        persist = stack.enter_context(tc.tile_pool(name="persist", bufs=1))
        wpool = stack.enter_context(tc.tile_pool(name="wpool", bufs=1))
        big = stack.enter_context(tc.tile_pool(name="big", bufs=1))

        # x first — it heads the critical path into norm1 + AG
        xm_sb = persist.tile([128, 4, D], F32)   # x, later mid (x + attn)
        nc.sync.dma_start(out=xm_sb, in_=x_in[:].rearrange("(t p) d -> p t d", p=128))

        ones_sb = consts.tile([1, 64], BF16)
        nc.vector.memset(ones_sb, 1.0)
        ident_sb = consts.tile([128, 128], BF16)
        nc.sync.dma_start(out=ident_sb, in_=ident_in[:])
        eps_sb = consts.tile([128, 1], F32)
        nc.vector.memset(eps_sb, EPS)

        def load_w(name, dram, cols):
            t = wpool.tile([128, 8, cols], BF16, tag=name)
            nc.scalar.dma_start(out=t, in_=dram[:].rearrange("(c p) f -> p c f", p=128))
            return t

        qT_sb = persist.tile([128, B * S], BF16)
        kT_sb = persist.tile([128, B * S], BF16)
        v_sb = persist.tile([128, 32, 130], BF16)
        oT_sb = persist.tile([128, B * S], BF16)

        # ---- norm helper: src [128,4,D] f32 -> tm/tin DRAM + AG ----
        def rmsnorm_to_ag(src_sb, tm_dram, tin_dram, ag_out_dram):
            with (
                tc.tile_pool(name="norm", bufs=2) as npool,
                tc.tile_pool(name="norm_ps", bufs=2, space="PSUM") as nps,
            ):
                hT_own = npool.tile([128, 8, T], BF16, tag="hT_own")
                for tt4 in range(4):
                    xsl = src_sb[:, tt4, :]
                    stats = npool.tile([128, 2, 6], F32, tag="stats")
                    nc.vector.bn_stats(out=stats[:, 0, :], in_=xsl[:, 0:512])
                    nc.vector.bn_stats(out=stats[:, 1, :], in_=xsl[:, 512:1024])
                    mv = npool.tile([128, 2], F32, tag="mv")
                    nc.vector.bn_aggr(out=mv, in_=stats)
                    msq = npool.tile([128, 1], F32, tag="msq")
                    nc.vector.tensor_mul(out=msq, in0=mv[:, 0:1], in1=mv[:, 0:1])
                    nc.vector.tensor_add(out=msq, in0=msq, in1=mv[:, 1:2])
                    rstd = npool.tile([128, 1], F32, tag="rstd")
                    nc.scalar.activation(out=rstd, in_=msq, func=AF.Sqrt, bias=eps_sb)
                    nc.vector.reciprocal(out=rstd, in_=rstd)
                    h_t = npool.tile([128, D], BF16, tag="h_t")
                    nc.vector.tensor_scalar_mul(out=h_t, in0=xsl, scalar1=rstd)
                    # transpose on PE (idle here): h_t [128 t, D] -> hT chunks
                    for dch in range(8):
                        ps_t = nps.tile([128, 128], BF16, tag="ps_t")
                        nc.tensor.transpose(out=ps_t, in_=h_t[:, ts(dch, 128)],
                                            identity=ident_sb)
                        nc.vector.tensor_copy(out=hT_own[:, dch, ts(tt4, 128)],
                                              in_=ps_t)
                nc.sync.dma_start(
                    out=tin_dram[:].rearrange("(c p) t -> p c t", p=128), in_=hT_own)
            nc.gpsimd.collective_compute(
                "AllGather", mybir.AluOpType.bypass, replica_groups=rg,
                ins=[tin_dram[:]], outs=[ag_out_dram[:]])

        def load_hT(ag_dram):
            hT = big.tile([128, 8, B * S], BF16, tag="big")
            for j in range(N):
                nc.sync.dma_start(
                    out=hT[:, :, ts(j, T)],
                    in_=ag_dram[ts(j, D), :].rearrange("(c p) t -> p c t", p=128))
            return hT

        # ================= phase 1: norm1 + AG =================
        rmsnorm_to_ag(xm_sb, h1_tm, h1t_in, h1t_ag)

        # weight/const loads land here: they execute during the AllGather
        trigC_sb = consts.tile([128, S], F32)
        nc.sync.dma_start(out=trigC_sb, in_=trigC[:])
        trigS_sb = consts.tile([128, S], F32)
        nc.sync.dma_start(out=trigS_sb, in_=trigS[:])
        mask_sb = consts.tile([128, 128], BF16)
        nc.sync.dma_start(out=mask_sb, in_=mask_in[:])
        perm_sb = consts.tile([128, 128], BF16)
        nc.sync.dma_start(out=perm_sb, in_=perm_in[:])
        wqT_sb = load_w("wqT", wqT, 128)
        wkT_sb = load_w("wkT", wkT, 128)
        wvT_sb = load_w("wvT", wvT, 128)
        woT_sb = load_w("woT", woT, D)
        w1T_sb = load_w("w1T", w1T, DFFC)
        w3T_sb = load_w("w3T", w3T, DFFC)

        hT_sb = load_hT(h1t_ag)

        # ================= phase 2: QKV + RoPE =================
        with (
            tc.tile_pool(name="qkv_ps", bufs=2, space="PSUM") as qkv_ps,
            tc.tile_pool(name="rope", bufs=2) as rope,
        ):
            for tt in range(8):
                pos = (tt % 4) * 512
                for dst_sb, wT_t in ((qT_sb, wqT_sb), (kT_sb, wkT_sb)):
                    ps_x = qkv_ps.tile([128, 512], F32, tag="psx")
                    for dch in range(8):
                        nc.tensor.matmul(out=ps_x, lhsT=wT_t[:, dch, :],
                                         rhs=hT_sb[:, dch, ts(tt, 512)],
                                         start=dch == 0, stop=dch == 7)
                    # swapped-rows copy via permutation matmul (E<->O halves)
                    x_bf = rope.tile([128, 512], BF16, tag="x_bf")
                    nc.vector.tensor_copy(out=x_bf, in_=ps_x)
                    ps_xs = qkv_ps.tile([128, 512], F32, tag="psxs")
                    nc.tensor.matmul(out=ps_xs, lhsT=perm_sb, rhs=x_bf,
                                     start=True, stop=True)
                    t1 = rope.tile([128, 512], F32, tag="r1")
                    nc.vector.tensor_mul(out=t1, in0=ps_x,
                                         in1=trigC_sb[:, ds(pos, 512)])
                    t2 = rope.tile([128, 512], F32, tag="r2")
                    nc.vector.tensor_mul(out=t2, in0=ps_xs,
                                         in1=trigS_sb[:, ds(pos, 512)])
                    nc.vector.tensor_add(out=dst_sb[:, ts(tt, 512)], in0=t1, in1=t2)
                for st in range(4):
                    tg = tt * 4 + st
                    ps_v = qkv_ps.tile([128, 128], F32, tag="psv")
                    for dch in range(8):
                        nc.tensor.matmul(out=ps_v,
                                         lhsT=hT_sb[:, dch, ds(tt * 512 + st * 128, 128)],
                                         rhs=wvT_sb[:, dch, :],
                                         start=dch == 0, stop=dch == 7)
                    nc.vector.tensor_copy(out=v_sb[:, tg, 0:64], in_=ps_v[:, 0:64])
                    nc.vector.tensor_copy(out=v_sb[:, tg, 65:129], in_=ps_v[:, 64:128])
            nc.vector.memset(v_sb[:, :, 64:65], 1.0)
            nc.vector.memset(v_sb[:, :, 129:130], 1.0)

        # ================= phase 3: attention =================
        with (
            tc.tile_pool(name="attn_ps", bufs=2, space="PSUM") as attn_ps,
            tc.tile_pool(name="attn_sb", bufs=3) as attn_sb,
        ):
            for b in range(B):
                for h in range(HPC):
                    fr = 64 * h
                    vcol = 65 * h
                    for qt in range(4):
                        qbase = b * S + qt * 512
                        ps_o = attn_ps.tile([65, 512], F32, tag="ps_o")
                        nkt = 4 * qt + 4
                        for kt in range(nkt):
                            d_off = kt * 128 - qt * 512
                            c0 = max(d_off, 0)
                            ps_s = attn_ps.tile([128, 512], F32, tag="ps_s")
                            nc.tensor.matmul(
                                out=ps_s[:, c0:512],
                                lhsT=kT_sb[fr:fr + 64, ds(b * S + kt * 128, 128)],
                                rhs=qT_sb[fr:fr + 64, ds(qbase + c0, 512 - c0)],
                                start=True, stop=True)
                            pT = attn_sb.tile([128, 512], BF16, tag="pT")
                            nc.scalar.activation(out=pT[:, c0:512], in_=ps_s[:, c0:512],
                                                 func=AF.Exp)
                            if d_off >= 0:
                                nc.vector.tensor_mul(
                                    out=pT[:, ds(d_off, 128)],
                                    in0=pT[:, ds(d_off, 128)], in1=mask_sb)
                            nc.tensor.matmul(
                                out=ps_o[:, c0:512],
                                lhsT=v_sb[:, b * 16 + kt, vcol:vcol + 65],
                                rhs=pT[:, c0:512],
                                start=kt == 0, stop=kt == nkt - 1)
                        rec = attn_sb.tile([1, 512], F32, tag="rec")
                        nc.vector.reciprocal(out=rec, in_=ps_o[64:65, :])
                        rec_bf = attn_sb.tile([1, 512], BF16, tag="rec_bf")
                        nc.vector.tensor_copy(out=rec_bf, in_=rec)
                        ps_b = attn_ps.tile([64, 512], F32, tag="ps_b")
                        nc.tensor.matmul(out=ps_b, lhsT=ones_sb, rhs=rec_bf,
                                         start=True, stop=True)
                        bc_sb = attn_sb.tile([64, 512], F32, tag="bc")
                        nc.vector.tensor_copy(out=bc_sb, in_=ps_b)
                        nc.vector.tensor_mul(out=oT_sb[fr:fr + 64, ds(qbase, 512)],
                                             in0=ps_o[0:64, :], in1=bc_sb)

        # ================= phase 4: A2A of head outputs =================
        for j in range(N):
            nc.sync.dma_start(out=o_a2a_in[ts(j, 128), :], in_=oT_sb[:, ts(j, T)])
        nc.gpsimd.collective_compute(
            "AllToAll", mybir.AluOpType.bypass, replica_groups=rg,
            ins=[o_a2a_in[:]], outs=[o_a2a_out[:]])
        oag_sb = persist.tile([128, 8, T], BF16)
        nc.sync.dma_start(out=oag_sb,
                          in_=o_a2a_out[:].rearrange("(c p) t -> p c t", p=128))

        # ================= phase 5: o-proj + residual =================
        with tc.tile_pool(name="op_ps", bufs=2, space="PSUM") as op_ps:
            for tc4 in range(4):
                for n in range(2):
                    ps_op = op_ps.tile([128, 512], F32, tag="ps_op")
                    for fch in range(8):
                        nc.tensor.matmul(out=ps_op,
                                         lhsT=oag_sb[:, fch, ts(tc4, 128)],
                                         rhs=woT_sb[:, fch, ts(n, 512)],
                                         start=fch == 0, stop=fch == 7)
                    nc.vector.tensor_add(out=xm_sb[:, tc4, ts(n, 512)],
                                         in0=xm_sb[:, tc4, ts(n, 512)], in1=ps_op)

        # ================= phase 6: norm2 + AG =================
        rmsnorm_to_ag(xm_sb, h2_tm, h2t_in, h2t_ag)
        hT2_sb = load_hT(h2t_ag)

        # ================= phase 7: FFN up + SwiGLU =================
        # dff-outer so the first half of s finishes early and its AllToAll
        # overlaps the second half's compute.
        with (
            tc.tile_pool(name="ffn_ps", bufs=2, space="PSUM") as ffn_ps,
            tc.tile_pool(name="ffn_sb", bufs=3) as ffn_sb,
        ):
            for dc in range(4):
                for tt in range(8):
                    ps_u = ffn_ps.tile([128, 512], F32, tag="ps_u")
                    for dch in range(8):
                        nc.tensor.matmul(out=ps_u,
                                         lhsT=w1T_sb[:, dch, ts(dc, 128)],
                                         rhs=hT2_sb[:, dch, ts(tt, 512)],
                                         start=dch == 0, stop=dch == 7)
                    ps_g = ffn_ps.tile([128, 512], F32, tag="ps_g")
                    for dch in range(8):
                        nc.tensor.matmul(out=ps_g,
                                         lhsT=w3T_sb[:, dch, ts(dc, 128)],
                                         rhs=hT2_sb[:, dch, ts(tt, 512)],
                                         start=dch == 0, stop=dch == 7)
                    silu_t = ffn_sb.tile([128, 512], F32, tag="silu")
                    nc.scalar.activation(out=silu_t, in_=ps_u, func=AF.Silu)
                    s_t = ffn_sb.tile([128, 512], BF16, tag="s_t")
                    nc.vector.tensor_mul(out=s_t, in0=silu_t, in1=ps_g)
                    s_in = s_a2a_in_a if dc < 2 else s_a2a_in_b
                    nc.sync.dma_start(
                        out=s_in[ds(tt * 256 + (dc % 2) * 128, 128), :], in_=s_t)
                if dc == 1:
                    nc.gpsimd.collective_compute(
                        "AllToAll", mybir.AluOpType.bypass, replica_groups=rg,
                        ins=[s_a2a_in_a[:]], outs=[s_a2a_out_a[:]])
        nc.gpsimd.collective_compute(
            "AllToAll", mybir.AluOpType.bypass, replica_groups=rg,
            ins=[s_a2a_in_b[:]], outs=[s_a2a_out_b[:]])

        # ================= phase 8: down-proj + residual =================
        # sT chunk order: a-half chunks (j*2+c2) then b-half; w2T rows are
        # host-permuted to match.
        sT_sb = big.tile([128, 32, T], BF16, tag="big")
        nc.sync.dma_start(out=sT_sb[:, 0:16, :],
                          in_=s_a2a_out_a[:].rearrange("(c p) t -> p c t", p=128))
        nc.sync.dma_start(out=sT_sb[:, 16:32, :],
                          in_=s_a2a_out_b[:].rearrange("(c p) t -> p c t", p=128))
        with (
            tc.tile_pool(name="dn_ps", bufs=1, space="PSUM") as dn_ps,
            tc.tile_pool(name="dn_sb", bufs=8) as dn_sb,
        ):
            ps_d = [dn_ps.tile([128, 512], F32, tag=f"ps_d{i}", name=f"ps_d{i}")
                    for i in range(8)]
            for dc in range(32):
                for n in range(2):
                    w2c = dn_sb.tile([128, 512], BF16, tag=f"w2c{n}", name="w2c")
                    nc.scalar.dma_start(out=w2c, in_=w2T[ts(dc, 128), ts(n, 512)])
                    for tc4 in range(4):
                        nc.tensor.matmul(out=ps_d[n * 4 + tc4],
                                         lhsT=sT_sb[:, dc, ts(tc4, 128)],
                                         rhs=w2c,
                                         start=dc == 0, stop=dc == 31)
            for n in range(2):
                for tc4 in range(4):
                    o_t = dn_sb.tile([128, 512], F32, tag="o_t")
                    nc.vector.tensor_add(out=o_t, in0=xm_sb[:, tc4, ts(n, 512)],
                                         in1=ps_d[n * 4 + tc4])
                    nc.sync.dma_start(
                        out=out_ext[:].rearrange("(t p) d -> p t d", p=128)[:, tc4, ts(n, 512)],
                        in_=o_t)

    nc.compile()
    return nc


def _host_prep(inputs):
    x = np.asarray(inputs["x"], np.float32).reshape(B * S, D)
    w_q = np.asarray(inputs["w_q"], np.float32)
    w_k = np.asarray(inputs["w_k"], np.float32)
    w_v = np.asarray(inputs["w_v"], np.float32)
    w_o = np.asarray(inputs["w_o"], np.float32)
    ln1 = np.asarray(inputs["ln1_w"], np.float32)
    ln2 = np.asarray(inputs["ln2_w"], np.float32)
    w1 = np.asarray(inputs["w1"], np.float32)
    w2 = np.asarray(inputs["w2"], np.float32)
    w3 = np.asarray(inputs["w3"], np.float32)

    wq_f = (w_q * ln1[None, :]) / np.sqrt(DK)
    wk_f = w_k * ln1[None, :]
    wv_f = w_v * ln1[None, :]
    w1_f = w1 * ln2[None, :]
    w3_f = w3 * ln2[None, :]

    # RoPE feature permutation: per core rows [h0E, h0O, h1E, h1O]
    jj = np.arange(32)
    swap_rows = np.concatenate([jj + 32, jj, jj + 96, jj + 64])
    # perm matmul matrix: out[m] = in[swap_rows[m]] -> P[k, m] = 1 iff k = swap(m)
    perm_mat = np.zeros((128, 128), dtype=BF16NP)
    perm_mat[swap_rows, np.arange(128)] = 1.0

    inv_freq = THETA ** (-(np.arange(0, DK, 2, dtype=np.float32) / DK))
    t_pos = np.arange(S, dtype=np.float32)
    ang = inv_freq[:, None] * t_pos[None, :]          # [32, S]
    c32, s32 = np.cos(ang), np.sin(ang)
    trigC = np.concatenate([c32, c32, c32, c32]).astype(np.float32)
    trigS = np.concatenate([-s32, s32, -s32, s32]).astype(np.float32)

    ident = np.eye(128, dtype=BF16NP)
    k_idx = np.arange(128)[:, None]
    q_idx = np.arange(128)[None, :]
    mask = (q_idx >= k_idx).astype(BF16NP)

    woT = np.ascontiguousarray(w_o.T).astype(BF16NP)
    # w2T rows ordered to match the consumer's split-A2A chunk order:
    # a-half (j, c2) -> global rows j*512 + c2*128, then b-half (+256)
    row_order = []
    for half in range(2):
        for j in range(N):
            for c2 in range(2):
                base = j * DFFC + half * 256 + c2 * 128
                row_order.extend(range(base, base + 128))
    w2T = np.ascontiguousarray(w2.T[np.array(row_order)]).astype(BF16NP)

    in_maps = []
    for i in range(N):
        perm = []
        for h in range(HPC):
            base = (HPC * i + h) * DK
            perm.extend(base + 2 * jj)       # even
            perm.extend(base + 2 * jj + 1)   # odd
        perm = np.array(perm)
        wq_p = wq_f[perm]                    # [128, 1024]
        wk_p = wk_f[perm]
        wqT = np.ascontiguousarray(wq_p.T).astype(BF16NP)
        wkT = np.ascontiguousarray(wk_p.T).astype(BF16NP)
        in_maps.append({
            "x": np.ascontiguousarray(x[i * T:(i + 1) * T]),
            "wqT": wqT,
            "wkT": wkT,
            "rope_perm": perm_mat,
            "ident": ident,
            "wvT": np.ascontiguousarray(wv_f[i * 128:(i + 1) * 128].T).astype(BF16NP),
            "woT": woT,
            "w1T": np.ascontiguousarray(w1_f[i * DFFC:(i + 1) * DFFC].T).astype(BF16NP),
            "w3T": np.ascontiguousarray(w3_f[i * DFFC:(i + 1) * DFFC].T).astype(BF16NP),
            "w2T": w2T,
            "trigC": trigC,
            "trigS": trigS,
            "mask": mask,
        })
    return in_maps


def _get_nc():
    global _NC
    if _NC is None:
        _NC = _build()
    return _NC


def run(inputs, trace=False, **kw):
    nc = _get_nc()
    in_maps = _host_prep(inputs)
    res = run_bass_kernel_spmd(nc, in_maps, list(range(N)), trace=trace, **kw)
    out = np.concatenate([res.results[i]["out"] for i in range(N)], axis=0)
    return out.reshape(B, S, D).astype(np.float32), res


def kernel(**inputs):
    out, _ = run(inputs)
    return out



# revision 23
# speedup vs baseline: 1.3157x; 1.3157x over previous
"""Trainium2 8-core tensor-parallel transformer block (RMSNorm + RoPE causal
attention + SwiGLU FFN).

v3 design:
  - token-local everything except attention: each core owns 512 tokens.
  - QKV projected locally (all heads, own tokens), then ONE AllToAll (fp8,
    1.5MB) swaps token-sharding for head-sharding.
  - attention head-sharded (2 heads/core, all 4096 tokens); scores computed
    directly from fp8 q/k (x16 wire scale, descaled inside the exp); softmax
    denominator via a ones-column appended to V (free 65th PSUM row);
    kt-lookahead so AV(kt-1) keeps the PE busy while ACT runs exp(kt).
  - A2A of normalized head outputs (fp8 x16) back to token-sharding, then
    o-proj, norm2 and the whole SwiGLU FFN are token-local in bf16
    (w1/w3 slab-streamed, w2 chunk-streamed). No AllGathers.
  - precision mix (chosen against a numpy fp8 simulation of this exact
    pipeline): fp8 e4m3 for h1/wq/wk (DoubleRow q/k projections) and all
    collective wires; bf16 for the v/o/FFN matmuls.
  - PE kept warm across collective waits with dependency-chained filler
    matmuls (HAM re-throttles to 1.2GHz after ~3.4us idle otherwise).
  - softmax reciprocals batched into one [16,512] op via a DRAM bounce;
    per-(b,qt) broadcast via a selector matmul.
"""

from contextlib import ExitStack

import numpy as np
import ml_dtypes

import concourse.mybir as mybir
import concourse.tile as tile
from concourse import bacc
from concourse.bass import ds, ts
from concourse.bass_utils import run_bass_kernel_spmd

B, S, D, H, DK, DFF = 2, 2048, 1024, 16, 64, 4096
THETA, EPS = 10000.0, 1e-5
N = 8            # cores
T = B * S // N   # tokens per core (512)
HPC = H // N     # heads per core (2)

F32 = mybir.dt.float32
BF16 = mybir.dt.bfloat16
F8 = mybir.dt.float8e4
BF16NP = ml_dtypes.bfloat16
F8NP = ml_dtypes.float8_e4m3
AF = mybir.ActivationFunctionType
DR = mybir.MatmulPerfMode.DoubleRow
MUL = mybir.AluOpType.mult
ADD = mybir.AluOpType.add

WS = 512.0             # wq/wk fp8 quantization scale
QK_WIRE = 16.0         # q_rot/k_rot carried as x16 in fp8
V_WIRE = 16.0          # v carried as x16 in fp8
EXP_SCALE = 1.0 / (QK_WIRE * QK_WIRE)  # descale scores inside exp
O_SCALE = 16.0         # attn output carried as o*16 in fp8
O_DESC = 1.0 / O_SCALE

FILL0, FILL1, FILL2 = 8, 36, 12  # filler links: start / qkv-A2A / o-A2A

_NC = None


def _build():
    nc = bacc.Bacc("TRN2", target_bir_lowering=False)

    # ---- I/O ----
    x_in = nc.dram_tensor("x", [T, D], F32, kind="ExternalInput")
    wqT = nc.dram_tensor("wqT", [D, D], F8, kind="ExternalInput")
    wkT = nc.dram_tensor("wkT", [D, D], F8, kind="ExternalInput")
    wvT = nc.dram_tensor("wvT", [D, D], BF16, kind="ExternalInput")
    woT = nc.dram_tensor("woT", [D, D], BF16, kind="ExternalInput")
    w1T = nc.dram_tensor("w1T", [D, DFF], BF16, kind="ExternalInput")
    w3T = nc.dram_tensor("w3T", [D, DFF], BF16, kind="ExternalInput")
    w2T = nc.dram_tensor("w2T", [DFF, D], BF16, kind="ExternalInput")
    trigC = nc.dram_tensor("trigC", [128, T], F32, kind="ExternalInput")
    trigS = nc.dram_tensor("trigS", [128, T], F32, kind="ExternalInput")
    mask_in = nc.dram_tensor("mask", [128, 128], BF16, kind="ExternalInput")
    ident_in = nc.dram_tensor("ident", [128, 128], BF16, kind="ExternalInput")
    perm_in = nc.dram_tensor("rope_perm", [128, 128], BF16, kind="ExternalInput")
    sel_in = nc.dram_tensor("sel", [16, 8 * 128], BF16, kind="ExternalInput")
    out_ext = nc.dram_tensor("out", [T, D], F32, kind="ExternalOutput")

    # ---- internal DRAM (collective bounce buffers) ----
    qkv_in = nc.dram_tensor("qkv_in", [3 * N * 128, T], F8)
    qkv_out = nc.dram_tensor("qkv_out", [3 * N * 128, T], F8)
    o_in = nc.dram_tensor("o_in", [N * 128, T], F8)
    o_out = nc.dram_tensor("o_out", [N * 128, T], F8)
    den_dram = nc.dram_tensor("den_bounce", [1, 16 * 512], F32)

    rg = [list(range(N))]

    with tile.TileContext(nc) as tc, ExitStack() as stack:
        consts = stack.enter_context(tc.tile_pool(name="consts", bufs=1))
        persist = stack.enter_context(tc.tile_pool(name="persist", bufs=1))
        wpool = stack.enter_context(tc.tile_pool(name="wpool", bufs=1))
        fill_stack = ExitStack()
        fill_ps = fill_stack.enter_context(
            tc.tile_pool(name="fill_ps", bufs=1, space="PSUM"))

        # x first — critical path into norm1
        xm_sb = persist.tile([128, 4, D], F32)   # x, later x + attn
        nc.sync.dma_start(out=xm_sb, in_=x_in[:].rearrange("(t p) d -> p t d", p=128))

        # constants + filler scratch
        scrA = consts.tile([128, 512], BF16)
        nc.vector.memset(scrA, 0.0)
        scrB = consts.tile([128, 512], BF16)
        nc.vector.memset(scrB, 0.0)
        eps_sb = consts.tile([128, 1], F32)
        nc.vector.memset(eps_sb, EPS)
        ident_sb = consts.tile([128, 128], BF16)
        nc.scalar.dma_start(out=ident_sb, in_=ident_in[:])
        mask_sb = consts.tile([128, 128], BF16)
        nc.scalar.dma_start(out=mask_sb, in_=mask_in[:])
        perm_sb = consts.tile([128, 128], BF16)
        nc.scalar.dma_start(out=perm_sb, in_=perm_in[:])
        sel_sb = consts.tile([16, 8, 128], BF16)
        nc.scalar.dma_start(out=sel_sb, in_=sel_in[:].rearrange("j (g m) -> j g m", g=8))
        trigC_sb = consts.tile([128, T], F32)
        nc.scalar.dma_start(out=trigC_sb, in_=trigC[:])
        trigS_sb = consts.tile([128, T], F32)
        nc.scalar.dma_start(out=trigS_sb, in_=trigS[:])

        def fillers(n):
            # PE keep-warm chain: 3 matmuls + 1 ACT link per ~1.1us
            for i in range(n):
                ps_f = fill_ps.tile([128, 512], F32, tag="fill")
                src = scrA if i % 2 == 0 else scrB
                dst = scrB if i % 2 == 0 else scrA
                for _ in range(3):
                    nc.tensor.matmul(out=ps_f, lhsT=ident_sb, rhs=src,
                                     start=True, stop=True)
                nc.scalar.activation(out=dst, in_=ps_f, func=AF.Copy)

        fillers(FILL0)

        # weight loads (stream in during norm1/qkv/A2A)
        def load_w(name, dram, dt):
            t = wpool.tile([128, 8, D], dt, tag=name)
            nc.gpsimd.dma_start(out=t, in_=dram[:].rearrange("(c p) f -> p c f", p=128))
            return t

        wqT_sb = load_w("wqT", wqT, F8)
        wkT_sb = load_w("wkT", wkT, F8)
        wvT_sb = load_w("wvT", wvT, BF16)
        woT_sb = load_w("woT", woT, BF16)

        h1T_sb = persist.tile([128, 8, T], F8)     # h1 fp8 (q/k DoubleRow)
        hTb_sb = persist.tile([128, 8, T], BF16, tag="hTbf")  # h1 bf16, later h2

        # ---- rmsnorm + transpose: xm [128,4,D] f32 -> hT(s) [128, 8 dch, T]
        def rmsnorm_T(dsts, tagsfx):
            with (
                tc.tile_pool(name="norm" + tagsfx, bufs=2) as npool,
                tc.tile_pool(name="nps" + tagsfx, bufs=2, space="PSUM") as nps,
            ):
                for tt4 in range(4):
                    xsl = xm_sb[:, tt4, :]
                    stats = npool.tile([128, 2, 6], F32, tag="stats")
                    nc.vector.bn_stats(out=stats[:, 0, :], in_=xsl[:, 0:512])
                    nc.vector.bn_stats(out=stats[:, 1, :], in_=xsl[:, 512:1024])
                    mv = npool.tile([128, 2], F32, tag="mv")
                    nc.vector.bn_aggr(out=mv, in_=stats)
                    msq = npool.tile([128, 1], F32, tag="msq")
                    nc.vector.tensor_mul(out=msq, in0=mv[:, 0:1], in1=mv[:, 0:1])
                    nc.vector.tensor_add(out=msq, in0=msq, in1=mv[:, 1:2])
                    rstd = npool.tile([128, 1], F32, tag="rstd")
                    nc.scalar.activation(out=rstd, in_=msq, func=AF.Sqrt, bias=eps_sb)
                    nc.vector.reciprocal(out=rstd, in_=rstd)
                    h_t = npool.tile([128, D], BF16, tag="h_t")
                    nc.vector.tensor_scalar_mul(out=h_t, in0=xsl, scalar1=rstd)
                    for dch in range(8):
                        ps_t = nps.tile([128, 128], BF16, tag="ps_t")
                        nc.tensor.transpose(out=ps_t, in_=h_t[:, ts(dch, 128)],
                                            identity=ident_sb)
                        for dst in dsts:
                            nc.vector.tensor_copy(out=dst[:, dch, ts(tt4, 128)],
                                                  in_=ps_t)

        # ================= phase 1: norm1 =================
        rmsnorm_T([h1T_sb, hTb_sb], "1")

        # ================= phase 2: QKV + RoPE (own tokens, all heads) ====
        with (
            tc.tile_pool(name="qkv_ps", bufs=2, space="PSUM") as qkv_ps,
            tc.tile_pool(name="xs_ps", bufs=2, space="PSUM") as xs_ps,
            tc.tile_pool(name="vps", bufs=1, space="PSUM") as vps,
            tc.tile_pool(name="rope", bufs=2) as rope,
            tc.tile_pool(name="vpool", bufs=1) as vpool,
        ):
            for oc in range(8):
                for qk, wT_t in ((0, wqT_sb), (1, wkT_sb)):
                    ps_x = qkv_ps.tile([128, T], F32, tag="psx")
                    for g in range(4):
                        nc.tensor.matmul(out=ps_x,
                                         lhsT=wT_t[:, ds(2 * g, 2), ts(oc, 128)],
                                         rhs=h1T_sb[:, ds(2 * g, 2), :],
                                         start=g == 0, stop=g == 3, perf_mode=DR)
                    x_bf = rope.tile([128, T], BF16, tag="x_bf")
                    nc.scalar.activation(out=x_bf, in_=ps_x, func=AF.Copy)
                    ps_xs = xs_ps.tile([128, T], F32, tag="psxs")
                    nc.tensor.matmul(out=ps_xs, lhsT=perm_sb, rhs=x_bf,
                                     start=True, stop=True)
                    t1 = rope.tile([128, T], F32, tag="r1")
                    nc.vector.tensor_mul(out=t1, in0=ps_x, in1=trigC_sb)
                    t2 = rope.tile([128, T], F32, tag="r2")
                    nc.vector.tensor_mul(out=t2, in0=ps_xs, in1=trigS_sb)
                    st = rope.tile([128, T], F8, tag="st")
                    nc.vector.tensor_add(out=st, in0=t1, in1=t2)
                    nc.sync.dma_start(out=qkv_in[ds(384 * oc + 128 * qk, 128), :],
                                      in_=st)
            # v token-major (bf16 matmul from the bf16 h transpose)
            v_own = vpool.tile([128, 4, D], F8, tag="v_own")
            for tc4 in range(4):
                ps_v = [vps.tile([128, 512], F32, tag="psv%d" % nn,
                                 name="psv%d" % nn) for nn in range(2)]
                for dch in range(8):
                    for nn in range(2):
                        nc.tensor.matmul(out=ps_v[nn],
                                         lhsT=hTb_sb[:, dch, ts(tc4, 128)],
                                         rhs=wvT_sb[:, dch, ts(nn, 512)],
                                         start=dch == 0, stop=dch == 7)
                for nn in range(2):
                    nc.vector.tensor_scalar_mul(out=v_own[:, tc4, ts(nn, 512)],
                                                in0=ps_v[nn], scalar1=V_WIRE)
            # send v: shard j region rows [384j+256, 384j+384):
            #   cols [0:256) = (tc, d 0:64) of dims 128j..  (head 2j)
            #   cols [256:512) = (tc, d 64:128)             (head 2j+1)
            for j in range(N):
                nc.sync.dma_start(
                    out=qkv_in[ds(384 * j + 256, 128), ds(0, 256)].rearrange(
                        "p (c d) -> p c d", c=4),
                    in_=v_own[:, :, ds(128 * j, 64)])
                nc.sync.dma_start(
                    out=qkv_in[ds(384 * j + 256, 128), ds(256, 256)].rearrange(
                        "p (c d) -> p c d", c=4),
                    in_=v_own[:, :, ds(128 * j + 64, 64)])

        # ================= phase 3: A2A(qkv) =================
        nc.gpsimd.collective_compute(
            "AllToAll", mybir.AluOpType.bypass, replica_groups=rg,
            ins=[qkv_in[:]], outs=[qkv_out[:]])
        fillers(FILL1)

        qT_sb = persist.tile([128, B * S], F8)
        kT_sb = persist.tile([128, B * S], F8)
        v8_sb = persist.tile([128, 32, 128], F8)
        v_sb = persist.tile([128, 32, 132], BF16)
        nc.vector.memset(v_sb[:, :, 64:65], 1.0)
        nc.vector.memset(v_sb[:, :, 130:131], 1.0)
        engs = [nc.sync, nc.scalar, nc.gpsimd]
        for j in range(N):
            engs[j % 3].dma_start(out=qT_sb[:, ts(j, T)],
                                  in_=qkv_out[ds(384 * j, 128), :])
            engs[(j + 1) % 3].dma_start(out=kT_sb[:, ts(j, T)],
                                        in_=qkv_out[ds(384 * j + 128, 128), :])
            engs[(j + 2) % 3].dma_start(
                out=v8_sb[:, ds(4 * j, 4), 0:64],
                in_=qkv_out[ds(384 * j + 256, 128), ds(0, 256)].rearrange(
                    "p (c d) -> p c d", c=4))
            engs[j % 3].dma_start(
                out=v8_sb[:, ds(4 * j, 4), 64:128],
                in_=qkv_out[ds(384 * j + 256, 128), ds(256, 256)].rearrange(
                    "p (c d) -> p c d", c=4))
        nc.vector.tensor_copy(out=v_sb[:, :, 0:64], in_=v8_sb[:, :, 0:64])
        nc.vector.tensor_copy(out=v_sb[:, :, 66:130], in_=v8_sb[:, :, 64:128])

        # ================= phase 4: attention =================
        oU_sb = persist.tile([128, B * S], BF16)   # unnormalized o (x V_WIRE)
        oT_sb = persist.tile([128, B * S], F8)     # normalized o (x O_SCALE)
        den_sb = persist.tile([16, 512], F32)
        with (
            tc.tile_pool(name="attn_s", bufs=2, space="PSUM") as attn_s,
            tc.tile_pool(name="attn_o", bufs=1, space="PSUM") as attn_o,
            tc.tile_pool(name="attn_sb", bufs=3) as attn_sb,
        ):
            for b in range(B):
                for qt in range(4):
                    qbase = b * S + qt * 512
                    nkt = 4 * qt + 4
                    ps_o = [attn_o.tile([65, 512], F32, tag="ps_o%d" % h,
                                        name="ps_o%d" % h) for h in range(HPC)]

                    def emit_av(kt, c0, pT):
                        for h in range(HPC):
                            nc.tensor.matmul(
                                out=ps_o[h][:, c0:512],
                                lhsT=v_sb[:, b * 16 + kt, ds(66 * h, 65)],
                                rhs=pT[:, h, c0:512],
                                start=kt == 0, stop=kt == nkt - 1)

                    prev = None
                    for kt in range(nkt):
                        d_off = kt * 128 - qt * 512
                        c0 = max(d_off, 0)
                        ps_s = attn_s.tile([128, 2, 512], F32, tag="ps_s")
                        pT = attn_sb.tile([128, 2, 512], BF16, tag="pT")
                        for h in range(HPC):
                            fr = 64 * h
                            nc.tensor.matmul(
                                out=ps_s[:, h, c0:512],
                                lhsT=kT_sb[fr:fr + 64, ds(b * S + kt * 128, 128)],
                                rhs=qT_sb[fr:fr + 64, ds(qbase + c0, 512 - c0)],
                                start=True, stop=True)
                        # AV(kt-1) keeps PE busy while ACT runs exp(kt)
                        if prev is not None:
                            emit_av(*prev)
                        nc.scalar.activation(out=pT[:, :, c0:512],
                                             in_=ps_s[:, :, c0:512], func=AF.Exp,
                                             scale=EXP_SCALE)
                        if d_off >= 0:
                            for h in range(HPC):
                                nc.vector.tensor_mul(
                                    out=pT[:, h, ds(d_off, 128)],
                                    in0=pT[:, h, ds(d_off, 128)], in1=mask_sb)
                        prev = (kt, c0, pT)
                    emit_av(*prev)
                    for h in range(HPC):
                        jrow = (b * 4 + qt) * 2 + h
                        nc.vector.tensor_copy(
                            out=oU_sb[ds(64 * h, 64), ds(qbase, 512)],
                            in_=ps_o[h][0:64, :])
                        dstg = attn_sb.tile([1, 512], F32, tag="dstg")
                        nc.vector.tensor_copy(out=dstg, in_=ps_o[h][64:65, :])
                        nc.sync.dma_start(
                            out=den_dram[0:1, ds(512 * jrow, 512)], in_=dstg)

        # batched softmax normalization
        with (
            tc.tile_pool(name="nrm_ps", bufs=2, space="PSUM") as nrm_ps,
            tc.tile_pool(name="nrm_sb", bufs=1) as nrm_sb,
        ):
            # redistribute 16 packed den rows onto 16 partitions via DRAM
            nc.sync.dma_start(
                out=den_sb, in_=den_dram[:].rearrange("o (j t) -> (o j) t", j=16))
            rec = nrm_sb.tile([16, 512], F32, tag="rec")
            nc.vector.reciprocal(out=rec, in_=den_sb)
            rec_bf = nrm_sb.tile([16, 512], BF16, tag="rec_bf")
            nc.vector.tensor_copy(out=rec_bf, in_=rec)
            for grp in range(8):
                qbase = grp * 512  # (b*4+qt)*512 == global col base
                ps_b = nrm_ps.tile([128, 512], F32, tag="ps_b")
                nc.tensor.matmul(out=ps_b, lhsT=sel_sb[:, grp, :], rhs=rec_bf,
                                 start=True, stop=True)
                nc.vector.tensor_mul(out=oT_sb[:, ds(qbase, 512)],
                                     in0=oU_sb[:, ds(qbase, 512)], in1=ps_b)

        # ================= phase 5: A2A(o) =================
        for j in range(N):
            eng = nc.sync if j % 2 == 0 else nc.scalar
            eng.dma_start(out=o_in[ts(j, 128), :], in_=oT_sb[:, ts(j, T)])
        fillers(6)   # cover the normalization/send chain
        nc.gpsimd.collective_compute(
            "AllToAll", mybir.AluOpType.bypass, replica_groups=rg,
            ins=[o_in[:]], outs=[o_out[:]])
        fillers(FILL2)
        oag_sb = persist.tile([128, 8, T], F8)
        nc.sync.dma_start(out=oag_sb,
                          in_=o_out[:].rearrange("(c p) t -> p c t", p=128))
        oagb_sb = persist.tile([128, 8, T], BF16)
        nc.vector.tensor_copy(out=oagb_sb, in_=oag_sb)

        # ================= phase 6: o-proj + residual =================
        with tc.tile_pool(name="op_ps", bufs=2, space="PSUM") as op_ps:
            for tc4 in range(4):
                ps_op = [op_ps.tile([128, 512], F32, tag="ps_op%d" % nn,
                                    name="ps_op%d" % nn) for nn in range(2)]
                for fch in range(8):
                    for nn in range(2):
                        nc.tensor.matmul(out=ps_op[nn],
                                         lhsT=oagb_sb[:, fch, ts(tc4, 128)],
                                         rhs=woT_sb[:, fch, ts(nn, 512)],
                                         start=fch == 0, stop=fch == 7)
                for nn in range(2):
                    nc.vector.scalar_tensor_tensor(
                        out=xm_sb[:, tc4, ts(nn, 512)], in0=ps_op[nn],
                        scalar=O_DESC, in1=xm_sb[:, tc4, ts(nn, 512)],
                        op0=MUL, op1=ADD)

        # fillers no longer needed; free the PSUM bank for down-proj
        fill_stack.close()

        # ================= phase 7: norm2 =================
        h2T_sb = persist.tile([128, 8, T], BF16, tag="hTbf")  # reuse h1 bf16 buf
        rmsnorm_T([h2T_sb], "2")

        # ================= phase 8: FFN up + SwiGLU (bf16) =================
        sT_sb = persist.tile([128, 32, 512], BF16)
        with (
            tc.tile_pool(name="ffn_ps", bufs=2, space="PSUM") as ffn_ps,
            tc.tile_pool(name="ffn_sb", bufs=2) as ffn_sb,
            tc.tile_pool(name="ffn_w", bufs=2) as ffn_w,
        ):
            for sl in range(16):
                w1c = ffn_w.tile([128, 8, 256], BF16, tag="w1c")
                nc.scalar.dma_start(
                    out=w1c, in_=w1T[:, ts(sl, 256)].rearrange("(c p) f -> p c f", p=128))
                w3c = ffn_w.tile([128, 8, 256], BF16, tag="w3c")
                nc.sync.dma_start(
                    out=w3c, in_=w3T[:, ts(sl, 256)].rearrange("(c p) f -> p c f", p=128))
                for fc2 in range(2):
                    fc = sl * 2 + fc2
                    ps_u = ffn_ps.tile([128, 512], F32, tag="ps_u")
                    ps_g = ffn_ps.tile([128, 512], F32, tag="ps_g")
                    for dch in range(8):
                        nc.tensor.matmul(out=ps_u,
                                         lhsT=w1c[:, dch, ts(fc2, 128)],
                                         rhs=h2T_sb[:, dch, :],
                                         start=dch == 0, stop=dch == 7)
                    for dch in range(8):
                        nc.tensor.matmul(out=ps_g,
                                         lhsT=w3c[:, dch, ts(fc2, 128)],
                                         rhs=h2T_sb[:, dch, :],
                                         start=dch == 0, stop=dch == 7)
                    silu_t = ffn_sb.tile([128, 512], F32, tag="silu")
                    nc.scalar.activation(out=silu_t, in_=ps_u, func=AF.Silu)
                    nc.vector.tensor_mul(out=sT_sb[:, fc, :], in0=silu_t, in1=ps_g)

        # ================= phase 9: down-proj + residual (bf16) ===========
        with (
            tc.tile_pool(name="dn_ps", bufs=1, space="PSUM") as dn_ps,
            tc.tile_pool(name="dn_w", bufs=6) as dn_w,
            tc.tile_pool(name="dn_sb", bufs=4) as dn_sb,
        ):
            ps_d = [dn_ps.tile([128, 512], F32, tag="ps_d%d" % i, name="ps_d%d" % i)
                    for i in range(8)]
            for dc in range(32):
                for nn in range(2):
                    w2c = dn_w.tile([128, 512], BF16, tag="w2c")
                    eng = nc.sync if (2 * dc + nn) % 2 == 0 else nc.scalar
                    eng.dma_start(out=w2c, in_=w2T[ts(dc, 128), ts(nn, 512)])
                    for tc4 in range(4):
                        nc.tensor.matmul(out=ps_d[nn * 4 + tc4],
                                         lhsT=sT_sb[:, dc, ts(tc4, 128)],
                                         rhs=w2c,
                                         start=dc == 0, stop=dc == 31)
            for nn in range(2):
                for tc4 in range(4):
                    o_t = dn_sb.tile([128, 512], F32, tag="o_t")
                    nc.vector.tensor_add(out=o_t, in0=xm_sb[:, tc4, ts(nn, 512)],
                                         in1=ps_d[nn * 4 + tc4])
                    nc.sync.dma_start(
                        out=out_ext[:].rearrange("(t p) d -> p t d", p=128)[:, tc4, ts(nn, 512)],
                        in_=o_t)

    nc.compile()
    return nc


def _host_prep(inputs):
    x = np.asarray(inputs["x"], np.float32).reshape(B * S, D)
    w_q = np.asarray(inputs["w_q"], np.float32)
    w_k = np.asarray(inputs["w_k"], np.float32)
    w_v = np.asarray(inputs["w_v"], np.float32)
    w_o = np.asarray(inputs["w_o"], np.float32)
    ln1 = np.asarray(inputs["ln1_w"], np.float32)
    ln2 = np.asarray(inputs["ln2_w"], np.float32)
    w1 = np.asarray(inputs["w1"], np.float32)
    w2 = np.asarray(inputs["w2"], np.float32)
    w3 = np.asarray(inputs["w3"], np.float32)

    def q8(a):  # quantize to TRN e4m3 (max +-240) at xWS
        return np.clip(a * WS, -240.0, 240.0).astype(F8NP)

    wq_f = (w_q * ln1[None, :]) / np.sqrt(DK)
    wk_f = w_k * ln1[None, :]
    wv_f = w_v * ln1[None, :]
    w1_f = w1 * ln2[None, :]
    w3_f = w3 * ln2[None, :]

    # RoPE row order: per head h: [even dims (32), odd dims (32)]
    jj = np.arange(32)
    perm_g = np.concatenate(
        [np.concatenate([64 * h + 2 * jj, 64 * h + 2 * jj + 1]) for h in range(H)])
    wqT8 = np.ascontiguousarray(q8(wq_f[perm_g].T))
    wkT8 = np.ascontiguousarray(q8(wk_f[perm_g].T))
    wvTb = np.ascontiguousarray(wv_f.T.astype(BF16NP))
    woTb = np.ascontiguousarray(w_o.T.astype(BF16NP))
    w1Tb = np.ascontiguousarray(w1_f.T.astype(BF16NP))
    w3Tb = np.ascontiguousarray(w3_f.T.astype(BF16NP))
    w2Tb = np.ascontiguousarray(w2.T.astype(BF16NP))

    # rope swap matrix (per 128-row chunk: E<->O halves within each head)
    swap_rows = np.concatenate([jj + 32, jj, jj + 96, jj + 64])
    perm_mat = np.zeros((128, 128), dtype=BF16NP)
    perm_mat[swap_rows, np.arange(128)] = 1.0

    inv_freq = THETA ** (-(np.arange(0, DK, 2, dtype=np.float32) / DK))

    ident = np.eye(128, dtype=BF16NP)
    k_idx = np.arange(128)[:, None]
    q_idx = np.arange(128)[None, :]
    mask = (q_idx >= k_idx).astype(BF16NP)

    # selector for softmax-denominator broadcast: [16, 8, 128]
    sel = np.zeros((16, 8, 128), dtype=BF16NP)
    for grp in range(8):
        sel[2 * grp, grp, 0:64] = 1.0
        sel[2 * grp + 1, grp, 64:128] = 1.0
    sel = sel.reshape(16, 8 * 128)

    # trig tables fold: psum is xWS (fp8 weights), wire target is xQK_WIRE
    tscale = QK_WIRE / WS
    in_maps = []
    for i in range(N):
        pos = (i % 4) * 512 + np.arange(T, dtype=np.float32)
        ang = inv_freq[:, None] * pos[None, :]          # [32, T]
        c32, s32 = np.cos(ang), np.sin(ang)
        trigC = (np.concatenate([c32, c32, c32, c32]) * tscale).astype(np.float32)
        trigS = (np.concatenate([-s32, s32, -s32, s32]) * tscale).astype(np.float32)
        in_maps.append({
            "x": np.ascontiguousarray(x[i * T:(i + 1) * T]),
            "wqT": wqT8,
            "wkT": wkT8,
            "wvT": wvTb,
            "woT": woTb,
            "w1T": w1Tb,
            "w3T": w3Tb,
            "w2T": w2Tb,
            "trigC": trigC,
            "trigS": trigS,
            "mask": mask,
            "ident": ident,
            "rope_perm": perm_mat,
            "sel": sel,
        })
    return in_maps


def _get_nc():
    global _NC
    if _NC is None:
        _NC = _build()
    return _NC


def run(inputs, trace=False, **kw):
    nc = _get_nc()
    in_maps = _host_prep(inputs)
    res = run_bass_kernel_spmd(nc, in_maps, list(range(N)), trace=trace, **kw)
    out = np.concatenate([res.results[i]["out"] for i in range(N)], axis=0)
    return out.reshape(B, S, D).astype(np.float32), res


def kernel(**inputs):
    out, _ = run(inputs)
    return out
